# revision 15
# baseline (speedup 1.0000x reference)
"""BinaryTreeLSTM on 8 Trainium2 NeuronCores (Bass/Tile).

Sharding: each core owns a contiguous subtree of 4096 leaves and reduces
it through the 4 largest internal levels (2048/1024/512/256 nodes per
core). The remaining global levels (1024 nodes down to the root, 2047
nodes = 0.25% of FLOPs) are latency-bound on device, so they are
finished on the host with BLAS from the per-core level-256 h/c — this
removes the AllGather collective and the serial small-level tail
entirely. Gate weights are replicated across cores.

Per-level node arrays are stored in bit-reversed node order on device so
each level's left/right children are the contiguous halves of the child
level; the host un-permutes when reassembling the natural level-order
output.

Device layout: hidden dim (256 = 2 chunks of 128) on the SBUF partition
axis, nodes on the free axis. Matmuls default to bf16 operands (fp32
PSUM accumulation; ~1e-3 output error) which halves input DMA and
enables fast weight loads; set TRNK_MM_DTYPE=float32r for the fp32r
path. Gate pre-activations use 2-bank PSUM tiles so one 1024-column
Scalar-engine activation (with the per-gate bias folded in) covers a
whole node chunk, amortizing the ~352-cycle per-instruction overhead.
"""

import os
import sys

import numpy as np

sys.path.insert(0, "/opt/trn_rl_repo")

HIDDEN = 256
NCORES = 8
CUT = 256       # smallest per-core level computed on device

# exposed for test harnesses
LAST_RESULTS = None
LAST_EXEC_NS = None
LAST_OPS = None


def _revperm(n):
    bits = n.bit_length() - 1
    r = np.arange(n)
    out = np.zeros(n, np.int64)
    for b in range(bits):
        out |= ((r >> b) & 1) << (bits - 1 - b)
    return out


def _w_tile_index(src, g, kc, hc):
    return ((src * 4 + g) * 2 + kc) * 2 + hc


def _round_fp32r(a):
    """Round fp32 values to the PE's fp32r format (1+8+11 bits, RNE)."""
    bits = np.ascontiguousarray(a, np.float32).view(np.uint32)
    odd = (bits >> np.uint32(12)) & np.uint32(1)
    bits = bits + np.uint32(0x7FF) + odd
    bits &= np.uint32(0xFFFFF000)
    return bits.view(np.float32)


def _pack_weights(Wx, Wl, Wr):
    # lhsT tile for (src, g, kc, hc): [p(contraction), m(out)] = W[g, hc*128+m, kc*128+p]
    tiles = []
    for W in (Wx, Wl, Wr):
        W4 = W.reshape(4, 2, 128, 2, 128)           # [g, hc, m, kc, p]
        tiles.append(W4.transpose(0, 3, 1, 4, 2))    # [g, kc, hc, p, m]
    allw = np.stack(tiles)                            # [3, 4, 2, 2, 128, 128]
    # -> [p, (s,g,kc,hc), m]
    blob = np.ascontiguousarray(allw.transpose(4, 0, 1, 2, 3, 5).reshape(128, 48, 128))
    return blob.astype(np.float32)


def _build_program(LPC, matmul_dtype_name="bfloat16"):
    from concourse import bacc, mybir, tile

    f32 = mybir.dt.float32
    mmdt = getattr(mybir.dt, matmul_dtype_name)
    bf16 = matmul_dtype_name == "bfloat16"
    sdt = mmdt if bf16 else f32
    CH = 512  # node-chunk (one z = CH fp32 = one PSUM bank)
    AF = mybir.ActivationFunctionType

    sizes = []
    n = LPC
    while n >= CUT:
        sizes.append(n)
        n //= 2
    offs = np.concatenate([[0], np.cumsum(sizes)]).astype(int)
    BIGTOT = int(offs[-3])          # leaves .. L1024 go to out_d
    NCHUNK = LPC // CH

    nc = bacc.Bacc("TRN2", target_bir_lowering=False, debug=False,
                   num_devices=NCORES)

    x_d = nc.dram_tensor("x", [128, NCHUNK, 2, CH], mmdt,
                         kind="ExternalInput").ap()
    wt_d = nc.dram_tensor("wt", [128, 48, 128], mmdt, kind="ExternalInput").ap()
    bias_d = nc.dram_tensor("bias", [128, 8], f32, kind="ExternalInput").ap()
    out_d = nc.dram_tensor("out", [2, 128, BIGTOT], sdt,
                           kind="ExternalOutput").ap()
    # h of the last two levels, flat [hc*768 + (L512 | L256)] per partition
    tail_d = nc.dram_tensor("tail", [128, 2, 768], sdt,
                            kind="ExternalOutput").ap()
    cend_d = nc.dram_tensor("cend", [128, 2, CUT], f32,
                            kind="ExternalOutput").ap()

    with tile.TileContext(nc) as tc:
        with tc.tile_pool(name="pp", bufs=1) as pp, \
             tc.tile_pool(name="zp", bufs=6, space="PSUM") as zp, \
             tc.tile_pool(name="gp", bufs=3) as gp:
            w_sb = pp.tile([128, 48, 128], mmdt, name="w_sb")
            bias_sb = pp.tile([128, 8], f32, name="bias_sb")
            hA = pp.tile([128, 2, LPC], sdt, name="hA")
            cA = pp.tile([128, 2, LPC], f32, name="cA")
            hB = pp.tile([128, 2, LPC // 2], sdt, name="hB")
            cB = pp.tile([128, 2, LPC // 2], f32, name="cB")
            x0_sb = pp.tile([128, 2, CH], mmdt, name="x0_sb")
            xr_sb = pp.tile([128, NCHUNK - 1, 2, CH], mmdt, name="xr_sb")
            tail_sb = pp.tile([128, 2, 768], sdt, name="tail_sb")
            cend_sb = pp.tile([128, 2, CUT], f32, name="cend_sb")

            # leaf weights + first x chunk go first so the Tensor/Scalar
            # engines start as early as possible; big single DMAs keep the
            # descriptor count low (DMA here is packet-rate-bound). Issued
            # from the Tensor/Vector queues because their sequencers boot
            # first (~2-4us) while Sync only comes up at ~10us.
            nc.scalar.dma_start(out=w_sb[:, 0:16, :], in_=wt_d[:, 0:16, :])
            nc.scalar.dma_start(out=x0_sb[:], in_=x_d[:, 0])
            nc.scalar.dma_start(out=bias_sb[:], in_=bias_d[:])
            nc.sync.dma_start(out=xr_sb[:], in_=x_d[:, 1:NCHUNK])
            nc.sync.dma_start(out=w_sb[:, 16:48, :], in_=wt_d[:, 16:48, :])

            if bf16:
                cast_rhs = lambda ap: ap  # noqa: E731
            else:
                cast_rhs = lambda ap: ap.bitcast(mmdt)  # noqa: E731

            def mm(w_idx, rhs_ap, zt, start, stop):
                nc.tensor.matmul(zt, w_sb[:, w_idx, :], cast_rhs(rhs_ap),
                                 start=start, stop=stop)

            def zmm(srcs, g, hc, zt, m):
                """z[:, :m] (+)= sum over (src, kc, h-ap); <=512 cols per mm."""
                for half in range(0, m, 512):
                    hw = min(512, m - half)
                    first = True
                    for src, kc, ap in srcs:
                        mm(_w_tile_index(src, g, kc, hc),
                           ap[:, half:half + hw],
                           zt[:, half:half + hw], first,
                           (src, kc) == srcs[-1][:2])
                        first = False

            def unit_internal(n, ch, h_src, c_src, h_dst, c_dst, lvl):
                """One chunk of an internal level -> (stage1, stage2)."""
                nchunks = max(1, n // CH)
                m = min(n, CH)
                lsl = slice(ch * m, (ch + 1) * m)
                rsl = slice(n + ch * m, n + (ch + 1) * m)
                dsl = slice(ch * m, (ch + 1) * m)
                i_t = gp.tile([128, 2, CH], f32, name="i_t")
                f_t = gp.tile([128, 2, CH], f32, name="f_t")
                o_t = gp.tile([128, 2, CH], f32, name="o_t")
                u_t = gp.tile([128, 2, CH], f32, name="u_t")
                gates = {0: i_t, 1: f_t, 2: o_t, 3: u_t}

                def s1():
                    # lc+rc goes straight into this chunk's c_dst slot
                    nc.gpsimd.tensor_add(c_dst[:, :, dsl],
                                         c_src[:, :, lsl],
                                         c_src[:, :, rsl])
                    for hc in range(2):
                        for g in (0, 3, 1, 2):
                            zt = zp.tile([128, CH], f32, name="zt")
                            zmm([(1, 0, h_src[:, 0, lsl]),
                                 (1, 1, h_src[:, 1, lsl]),
                                 (2, 0, h_src[:, 0, rsl]),
                                 (2, 1, h_src[:, 1, rsl])], g, hc, zt, m)
                            func = AF.Tanh if g == 3 else AF.Sigmoid
                            nc.scalar.activation(
                                out=gates[g][:, hc, :m], in_=zt[:, :m],
                                func=func,
                                bias=bias_sb[:, g * 2 + hc:g * 2 + hc + 1])

                def s2():
                    # hc-merged [128, 2, m] APs halve the instruction count
                    nc.vector.tensor_mul(u_t[:, :, :m], i_t[:, :, :m],
                                         u_t[:, :, :m])
                    nc.vector.tensor_mul(c_dst[:, :, dsl], f_t[:, :, :m],
                                         c_dst[:, :, dsl])
                    nc.vector.tensor_add(c_dst[:, :, dsl], u_t[:, :, :m],
                                         c_dst[:, :, dsl])
                    nc.scalar.activation(out=i_t[:, :, :m],
                                         in_=c_dst[:, :, dsl], func=AF.Tanh)
                    nc.vector.tensor_mul(h_dst[:, :, dsl],
                                         o_t[:, :, :m], i_t[:, :, :m])
                    if ch == nchunks - 1:
                        if n >= 1024:
                            for hc in range(2):
                                nc.sync.dma_start(
                                    out=out_d[hc, :, offs[lvl]:offs[lvl] + n],
                                    in_=h_dst[:, hc, :n])
                        else:
                            toff = 0 if n == 512 else 512
                            for hc in range(2):
                                nc.vector.tensor_copy(
                                    tail_sb[:, hc, toff:toff + n],
                                    h_dst[:, hc, :n])
                            if n == CUT:
                                for hc in range(2):
                                    nc.vector.tensor_copy(cend_sb[:, hc, :],
                                                          c_dst[:, hc, :n])
                                nc.sync.dma_start(out=tail_d[:], in_=tail_sb[:])
                                nc.sync.dma_start(out=cend_d[:], in_=cend_sb[:])
                return s1, s2

            def unit_leaves(ch):
                nsl = slice(ch * CH, (ch + 1) * CH)
                xc_t = x0_sb if ch == 0 else xr_sb[:, ch - 1]
                i_t = gp.tile([128, 2, CH], f32, name="i_t")
                o_t = gp.tile([128, 2, CH], f32, name="o_t")
                u_t = gp.tile([128, 2, CH], f32, name="u_t")
                lgates = {0: i_t, 2: o_t, 3: u_t}

                def s1():
                    for hc in range(2):
                        for g in (0, 3, 2):
                            zt = zp.tile([128, CH], f32, name="zt")
                            zmm([(0, 0, xc_t[:, 0]), (0, 1, xc_t[:, 1])],
                                g, hc, zt, CH)
                            func = AF.Tanh if g == 3 else AF.Sigmoid
                            nc.scalar.activation(
                                out=lgates[g][:, hc, :], in_=zt[:], func=func,
                                bias=bias_sb[:, g * 2 + hc:g * 2 + hc + 1])

                def s2():
                    nc.vector.tensor_mul(cA[:, :, nsl], i_t[:], u_t[:])
                    nc.scalar.activation(out=u_t[:], in_=cA[:, :, nsl],
                                         func=AF.Tanh)
                    nc.vector.tensor_mul(hA[:, :, nsl], o_t[:], u_t[:])
                    if ch == NCHUNK - 1:
                        for hc in range(2):
                            nc.sync.dma_start(out=out_d[hc, :, 0:LPC],
                                              in_=hA[:, hc, :])
                return s1, s2

            # ---- software-pipelined unit stream: leaves + internal levels ----
            units = [("leaf", ch, False) for ch in range(NCHUNK)]
            cur = [hA, cA, hB, cB]
            lvl = 1
            n = LPC // 2
            while n >= CUT:
                h_src, c_src, h_dst, c_dst = cur
                # if the child level had <= 2 chunks, this level's first s1
                # reads h written by a pending s2 -> must flush the pipeline
                flush = (2 * n) // CH <= 2
                for ch in range(max(1, n // CH)):
                    units.append(("int", (n, ch, h_src, c_src, h_dst, c_dst,
                                          lvl), flush and ch == 0))
                cur = [cur[2], cur[3], cur[0], cur[1]]
                lvl += 1
                n //= 2
            pending = []
            for u in units:
                if u[2]:
                    while pending:
                        pending.pop(0)()
                s1, s2 = (unit_leaves(u[1]) if u[0] == "leaf"
                          else unit_internal(*u[1]))
                s1()
                pending.append(s2)
                if len(pending) > 1:
                    pending.pop(0)()
            for s2 in pending:
                s2()

    nc.compile()
    return nc, sizes, offs


class _ExecHandle:
    """Compiled SPMD executable with device-resident input support."""

    def __init__(self, nc):
        import jax
        from jax.sharding import Mesh, PartitionSpec
        try:
            from jax.experimental.shard_map import shard_map
        except ImportError:
            from jax.shard_map import shard_map
        from concourse import bass2jax, mybir

        bass2jax.install_neuronx_cc_hook()
        self.jax = jax
        partition_name = (nc.partition_id_tensor.name
                          if nc.partition_id_tensor else None)
        in_names, out_names, out_avals, zero_outs = [], [], [], []
        for alloc in nc.m.functions[0].allocations:
            if not isinstance(alloc, mybir.MemoryLocationSet):
                continue
            name = alloc.memorylocations[0].name
            if alloc.kind == "ExternalInput":
                if name != partition_name:
                    in_names.append(name)
            elif alloc.kind == "ExternalOutput":
                out_names.append(name)
                shape = tuple(alloc.tensor_shape)
                dtype = mybir.dt.np(alloc.dtype)
                out_avals.append(jax.core.ShapedArray(shape, dtype))
                zero_outs.append(np.zeros(shape, dtype))
        self.n_params = len(in_names)
        self.out_names = list(out_names)
        self.param_names = list(in_names)
        all_in_names = in_names + out_names
        if partition_name is not None:
            all_in_names.append(partition_name)
        self.out_avals = out_avals
        self.zero_outs = zero_outs

        def _body(*args):
            operands = list(args)
            if partition_name is not None:
                operands.append(bass2jax.partition_id_tensor())
            outs = bass2jax._bass_exec_p.bind(
                *operands,
                out_avals=tuple(out_avals),
                in_names=tuple(all_in_names),
                out_names=tuple(out_names),
                lowering_input_output_aliases=(),
                sim_require_finite=True,
                sim_require_nnan=True,
                nc=nc,
            )
            return tuple(outs)

        self._body = _body

        devices = jax.devices()[:NCORES]
        self.mesh = Mesh(np.asarray(devices), ("core",))
        n_ops = self.n_params + len(out_names)
        self.fn = jax.jit(shard_map(
            _body, mesh=self.mesh,
            in_specs=(PartitionSpec("core"),) * n_ops,
            out_specs=(PartitionSpec("core"),) * len(out_names),
            check_rep=False))

    def put_inputs(self, in_maps):
        import jax
        from jax.sharding import NamedSharding, PartitionSpec
        sh = NamedSharding(self.mesh, PartitionSpec("core"))
        ops = []
        for i, name in enumerate(self.param_names):
            arr = np.concatenate([np.asarray(m[name]) for m in in_maps], axis=0)
            ops.append(jax.device_put(arr, sh))
        for z in self.zero_outs:
            zz = np.zeros((NCORES * z.shape[0], *z.shape[1:]), z.dtype)
            ops.append(jax.device_put(zz, sh))
        return ops

    def run(self, ops):
        outs = self.fn(*ops)
        self.jax.block_until_ready(outs)
        return outs

    def results(self, outs):
        res = []
        for c in range(NCORES):
            d = {}
            for i, name in enumerate(self.out_names):
                a = np.asarray(outs[i])
                d[name] = a.reshape(NCORES, *self.out_avals[i].shape)[c]
            res.append(d)
        return res


def _sigmoid(z):
    with np.errstate(over="ignore"):
        return 1.0 / (1.0 + np.exp(-z))


_PROGRAM_CACHE = {}
_EXEC_CACHE = {}


def kernel(tokens, emb, Wx, Wl, Wr, b):
    global LAST_RESULTS, LAST_OPS
    tokens = np.asarray(tokens)
    emb = np.asarray(emb, dtype=np.float32)
    Wx = np.asarray(Wx, dtype=np.float32)
    Wl = np.asarray(Wl, dtype=np.float32)
    Wr = np.asarray(Wr, dtype=np.float32)
    b = np.asarray(b, dtype=np.float32)

    L = int(tokens.shape[0])
    LPC = L // NCORES
    mmdt = os.environ.get("TRNK_MM_DTYPE", "bfloat16")
    key = (LPC, mmdt)
    if key not in _PROGRAM_CACHE:
        _PROGRAM_CACHE[key] = _build_program(LPC, mmdt)
    nc, sizes, offs = _PROGRAM_CACHE[key]
    CH = 512

    wt_blob = _pack_weights(Wx, Wl, Wr)
    bias_blob = np.ascontiguousarray(
        b.reshape(4, 2, 128).transpose(2, 0, 1).reshape(128, 8)).astype(np.float32)

    x = emb[tokens]  # [L, 256] host gather (input sharding/staging)
    rp = _revperm(LPC)
    if mmdt == "bfloat16":
        import ml_dtypes
        wt_blob = wt_blob.astype(ml_dtypes.bfloat16)
        cast = lambda a: a.astype(ml_dtypes.bfloat16)  # noqa: E731
    else:
        wt_blob = _round_fp32r(wt_blob)
        cast = _round_fp32r
    in_maps = []
    for ci in range(NCORES):
        xc = x[ci * LPC:(ci + 1) * LPC][rp]                   # stored order
        # [128, NCHUNK, 2, CH]: chunk-major so each chunk DMA has big lines
        xblob = np.ascontiguousarray(
            xc.reshape(LPC // CH, CH, 2, 128).transpose(3, 0, 2, 1))
        in_maps.append({"x": cast(xblob), "wt": wt_blob, "bias": bias_blob})

    if key not in _EXEC_CACHE:
        _EXEC_CACHE[key] = _ExecHandle(nc)
    eh = _EXEC_CACHE[key]
    ops = eh.put_inputs(in_maps)
    outs = eh.run(ops)
    results = eh.results(outs)
    LAST_RESULTS = results
    LAST_OPS = ops

    # ---- host reassembly of device levels (global 32768 .. 2048) ----
    pieces = []
    for lvl, npc in enumerate(sizes):
        nglob = npc * NCORES
        rpl = _revperm(npc)
        lvlarr = np.empty((nglob, HIDDEN), np.float32)
        for ci in range(NCORES):
            if lvl < len(sizes) - 2:
                st = results[ci]["out"][:, :, offs[lvl]:offs[lvl] + npc]
                st = st.reshape(HIDDEN, npc)
            else:
                toff = 0 if npc == 512 else 512
                st = results[ci]["tail"][:, :, toff:toff + npc]
                st = st.transpose(1, 0, 2).reshape(HIDDEN, npc)
            lvlarr[ci * npc:(ci + 1) * npc] = st.T[rpl].astype(np.float32)
        pieces.append(lvlarr)

    # ---- host tail: global levels 1024 .. 1 from per-core (h,c) at CUT ----
    rpc = _revperm(CUT)
    nglob = CUT * NCORES
    h = np.empty((nglob, HIDDEN), np.float32)
    c = np.empty((nglob, HIDDEN), np.float32)
    for ci in range(NCORES):
        st = results[ci]["tail"][:, :, 512:512 + CUT]
        h[ci * CUT:(ci + 1) * CUT] = (
            st.transpose(1, 0, 2).reshape(HIDDEN, CUT).T[rpc].astype(np.float32))
        stc = results[ci]["cend"]                       # [128, 2, CUT] f32
        c[ci * CUT:(ci + 1) * CUT] = (
            stc.transpose(1, 0, 2).reshape(HIDDEN, CUT).T[rpc])

    # y = x @ W.T per gate; stack gates on columns: [in, 4*out]
    WlT = np.ascontiguousarray(Wl.transpose(2, 0, 1).reshape(HIDDEN, 4 * HIDDEN))
    WrT = np.ascontiguousarray(Wr.transpose(2, 0, 1).reshape(HIDDEN, 4 * HIDDEN))
    bfl = b.reshape(4 * HIDDEN)
    while h.shape[0] > 1:
        lh, rh = h[0::2], h[1::2]
        lc, rc = c[0::2], c[1::2]
        z = lh @ WlT + rh @ WrT + bfl                   # [n, 4H]
        i = _sigmoid(z[:, 0 * HIDDEN:1 * HIDDEN])
        f = _sigmoid(z[:, 1 * HIDDEN:2 * HIDDEN])
        o = _sigmoid(z[:, 2 * HIDDEN:3 * HIDDEN])
        u = np.tanh(z[:, 3 * HIDDEN:4 * HIDDEN])
        c = i * u + f * (lc + rc)
        h = o * np.tanh(c)
        pieces.append(h)
    return np.concatenate(pieces, axis=0)


# revision 21
# speedup vs baseline: 1.0357x; 1.0357x over previous
"""BinaryTreeLSTM on 8 Trainium2 NeuronCores (Bass/Tile).

Sharding: each core owns a contiguous subtree of 4096 leaves and reduces
it through the 4 largest internal levels (2048/1024/512/256 nodes per
core). The remaining global levels (1024 nodes down to the root, 2047
nodes = 0.25% of FLOPs) are latency-bound on device, so they are
finished on the host with BLAS from the per-core level-256 h/c — this
removes the AllGather collective and the serial small-level tail
entirely. Gate weights are replicated across cores.

Per-level node arrays are stored in bit-reversed node order on device so
each level's left/right children are the contiguous halves of the child
level; the host un-permutes when reassembling the natural level-order
output.

Device layout: hidden dim (256 = 2 chunks of 128) on the SBUF partition
axis, nodes on the free axis. Matmuls default to bf16 operands (fp32
PSUM accumulation; ~1e-3 output error) which halves input DMA and
enables fast weight loads; set TRNK_MM_DTYPE=float32r for the fp32r
path. Gate pre-activations use 2-bank PSUM tiles so one 1024-column
Scalar-engine activation (with the per-gate bias folded in) covers a
whole node chunk, amortizing the ~352-cycle per-instruction overhead.
"""

import os
import sys

import numpy as np

sys.path.insert(0, "/opt/trn_rl_repo")

HIDDEN = 256
NCORES = 8
CUT = 256       # smallest per-core level computed on device

# exposed for test harnesses
LAST_RESULTS = None
LAST_EXEC_NS = None
LAST_OPS = None


def _revperm(n):
    bits = n.bit_length() - 1
    r = np.arange(n)
    out = np.zeros(n, np.int64)
    for b in range(bits):
        out |= ((r >> b) & 1) << (bits - 1 - b)
    return out


def _w_tile_index(src, g, kc, hc):
    return ((src * 4 + g) * 2 + kc) * 2 + hc


def _round_fp32r(a):
    """Round fp32 values to the PE's fp32r format (1+8+11 bits, RNE)."""
    bits = np.ascontiguousarray(a, np.float32).view(np.uint32)
    odd = (bits >> np.uint32(12)) & np.uint32(1)
    bits = bits + np.uint32(0x7FF) + odd
    bits &= np.uint32(0xFFFFF000)
    return bits.view(np.float32)


def _pack_weights(Wx, Wl, Wr):
    # lhsT tile for (src, g, kc, hc): [p(contraction), m(out)] = W[g, hc*128+m, kc*128+p]
    tiles = []
    for W in (Wx, Wl, Wr):
        W4 = W.reshape(4, 2, 128, 2, 128)           # [g, hc, m, kc, p]
        tiles.append(W4.transpose(0, 3, 1, 4, 2))    # [g, kc, hc, p, m]
    allw = np.stack(tiles)                            # [3, 4, 2, 2, 128, 128]
    # -> [p, (s,g,kc,hc), m]
    blob = np.ascontiguousarray(allw.transpose(4, 0, 1, 2, 3, 5).reshape(128, 48, 128))
    return blob.astype(np.float32)


def _build_program(LPC, matmul_dtype_name="bfloat16"):
    from concourse import bacc, mybir, tile

    f32 = mybir.dt.float32
    mmdt = getattr(mybir.dt, matmul_dtype_name)
    bf16 = matmul_dtype_name == "bfloat16"
    sdt = mmdt if bf16 else f32
    CH = 512  # node-chunk (one z = CH fp32 = one PSUM bank)
    AF = mybir.ActivationFunctionType

    sizes = []
    n = LPC
    while n >= CUT:
        sizes.append(n)
        n //= 2
    offs = np.concatenate([[0], np.cumsum(sizes)]).astype(int)
    BIGTOT = int(offs[-3])          # leaves .. L1024 go to out_d
    NCHUNK = LPC // CH

    nc = bacc.Bacc("TRN2", target_bir_lowering=False, debug=False,
                   num_devices=NCORES)

    x_d = nc.dram_tensor("x", [128, NCHUNK, 2, CH], mmdt,
                         kind="ExternalInput").ap()
    wt_d = nc.dram_tensor("wt", [128, 48, 128], mmdt, kind="ExternalInput").ap()
    bias_d = nc.dram_tensor("bias", [128, 8], f32, kind="ExternalInput").ap()
    out_d = nc.dram_tensor("out", [2, 128, BIGTOT], sdt,
                           kind="ExternalOutput").ap()
    # h of the last two levels, flat [hc*768 + (L512 | L256)] per partition
    tail_d = nc.dram_tensor("tail", [128, 2, 768], sdt,
                            kind="ExternalOutput").ap()
    cend_d = nc.dram_tensor("cend", [128, 2, CUT], f32,
                            kind="ExternalOutput").ap()

    with tile.TileContext(nc) as tc:
        with tc.tile_pool(name="pp", bufs=1) as pp, \
             tc.tile_pool(name="zp", bufs=8, space="PSUM") as zp, \
             tc.tile_pool(name="gp", bufs=4) as gp:
            w_sb = pp.tile([128, 48, 128], mmdt, name="w_sb")
            bias_sb = pp.tile([128, 8], f32, name="bias_sb")
            hA = pp.tile([128, 2, LPC], sdt, name="hA")
            cA = pp.tile([128, 2, LPC], f32, name="cA")
            hB = pp.tile([128, 2, LPC // 2], sdt, name="hB")
            cB = pp.tile([128, 2, LPC // 2], f32, name="cB")
            x0_sb = pp.tile([128, 2, CH], mmdt, name="x0_sb")
            xr_sb = pp.tile([128, NCHUNK - 1, 2, CH], mmdt, name="xr_sb")
            tail_sb = pp.tile([128, 2, 768], sdt, name="tail_sb")
            cend_sb = pp.tile([128, 2, CUT], f32, name="cend_sb")

            # Inputs split across the 3 DMA-capable engine queues (one DGE
            # queue per engine — same-engine DMAs serialize). First-needed
            # pieces (leaf weights, x chunk 0) lead their queues; big single
            # DMAs keep the descriptor count low (packet-rate-bound).
            nc.scalar.dma_start(out=w_sb[:, 0:16, :], in_=wt_d[:, 0:16, :])
            nc.scalar.dma_start(out=bias_sb[:], in_=bias_d[:])
            nc.sync.dma_start(out=x0_sb[:], in_=x_d[:, 0])
            nc.sync.dma_start(out=xr_sb[:], in_=x_d[:, 1:NCHUNK])
            nc.gpsimd.dma_start(out=w_sb[:, 16:48, :], in_=wt_d[:, 16:48, :])

            if bf16:
                cast_rhs = lambda ap: ap  # noqa: E731
            else:
                cast_rhs = lambda ap: ap.bitcast(mmdt)  # noqa: E731

            def mm(w_idx, rhs_ap, zt, start, stop):
                nc.tensor.matmul(zt, w_sb[:, w_idx, :], cast_rhs(rhs_ap),
                                 start=start, stop=stop)

            def zmm(srcs, g, hc, zt, m):
                """z[:, :m] (+)= sum over (src, kc, h-ap); <=512 cols per mm."""
                for half in range(0, m, 512):
                    hw = min(512, m - half)
                    first = True
                    for src, kc, ap in srcs:
                        mm(_w_tile_index(src, g, kc, hc),
                           ap[:, half:half + hw],
                           zt[:, half:half + hw], first,
                           (src, kc) == srcs[-1][:2])
                        first = False

            def unit_internal(n, ch, h_src, c_src, h_dst, c_dst, lvl):
                """One chunk of an internal level -> (stage1, stage2)."""
                nchunks = max(1, n // CH)
                m = min(n, CH)
                lsl = slice(ch * m, (ch + 1) * m)
                rsl = slice(n + ch * m, n + (ch + 1) * m)
                dsl = slice(ch * m, (ch + 1) * m)
                merge = n >= 2048  # hc-merged ops off the flush critical path
                i_t = gp.tile([128, 2, CH], f32, name="i_t")
                f_t = gp.tile([128, 2, CH], f32, name="f_t")
                o_t = gp.tile([128, 2, CH], f32, name="o_t")
                u_t = gp.tile([128, 2, CH], f32, name="u_t")
                s_t = gp.tile([128, 2, CH], f32, name="s_t")
                gates = {0: i_t, 1: f_t, 2: o_t, 3: u_t}

                def s1():
                    nc.gpsimd.tensor_add(s_t[:, :, :m],
                                         c_src[:, :, lsl],
                                         c_src[:, :, rsl])
                    for hc in range(2):
                        for g in (0, 3, 1, 2):
                            zt = zp.tile([128, CH], f32, name="zt")
                            zmm([(1, 0, h_src[:, 0, lsl]),
                                 (1, 1, h_src[:, 1, lsl]),
                                 (2, 0, h_src[:, 0, rsl]),
                                 (2, 1, h_src[:, 1, rsl])], g, hc, zt, m)
                            func = AF.Tanh if g == 3 else AF.Sigmoid
                            nc.scalar.activation(
                                out=gates[g][:, hc, :m], in_=zt[:, :m],
                                func=func,
                                bias=bias_sb[:, g * 2 + hc:g * 2 + hc + 1])

                def s2():
                    if merge:
                        # hc-merged [128, 2, m] APs halve the instruction count
                        nc.vector.tensor_mul(u_t[:, :, :m], i_t[:, :, :m],
                                             u_t[:, :, :m])
                        nc.vector.tensor_mul(s_t[:, :, :m], f_t[:, :, :m],
                                             s_t[:, :, :m])
                        nc.vector.tensor_add(c_dst[:, :, dsl], u_t[:, :, :m],
                                             s_t[:, :, :m])
                        nc.scalar.activation(out=i_t[:, :, :m],
                                             in_=c_dst[:, :, dsl],
                                             func=AF.Tanh)
                        nc.vector.tensor_mul(h_dst[:, :, dsl],
                                             o_t[:, :, :m], i_t[:, :, :m])
                    else:
                        # per-hc keeps two short parallel chains for the
                        # latency-critical flushed levels
                        for hc in range(2):
                            nc.vector.tensor_mul(u_t[:, hc, :m],
                                                 i_t[:, hc, :m],
                                                 u_t[:, hc, :m])
                            nc.vector.tensor_mul(s_t[:, hc, :m],
                                                 f_t[:, hc, :m],
                                                 s_t[:, hc, :m])
                            nc.vector.tensor_add(c_dst[:, hc, dsl],
                                                 u_t[:, hc, :m],
                                                 s_t[:, hc, :m])
                            nc.scalar.activation(out=i_t[:, hc, :m],
                                                 in_=c_dst[:, hc, dsl],
                                                 func=AF.Tanh)
                            nc.vector.tensor_mul(h_dst[:, hc, dsl],
                                                 o_t[:, hc, :m],
                                                 i_t[:, hc, :m])
                    if ch == nchunks - 1:
                        if n >= 1024:
                            for hc in range(2):
                                nc.sync.dma_start(
                                    out=out_d[hc, :, offs[lvl]:offs[lvl] + n],
                                    in_=h_dst[:, hc, :n])
                        else:
                            toff = 0 if n == 512 else 512
                            for hc in range(2):
                                nc.vector.tensor_copy(
                                    tail_sb[:, hc, toff:toff + n],
                                    h_dst[:, hc, :n])
                            if n == CUT:
                                for hc in range(2):
                                    nc.vector.tensor_copy(cend_sb[:, hc, :],
                                                          c_dst[:, hc, :n])
                                nc.sync.dma_start(out=tail_d[:], in_=tail_sb[:])
                                nc.sync.dma_start(out=cend_d[:], in_=cend_sb[:])
                return s1, s2

            def unit_leaves(ch):
                nsl = slice(ch * CH, (ch + 1) * CH)
                xc_t = x0_sb if ch == 0 else xr_sb[:, ch - 1]
                i_t = gp.tile([128, 2, CH], f32, name="i_t")
                o_t = gp.tile([128, 2, CH], f32, name="o_t")
                u_t = gp.tile([128, 2, CH], f32, name="u_t")
                lgates = {0: i_t, 2: o_t, 3: u_t}

                def s1():
                    for hc in range(2):
                        for g in (0, 3, 2):
                            zt = zp.tile([128, CH], f32, name="zt")
                            zmm([(0, 0, xc_t[:, 0]), (0, 1, xc_t[:, 1])],
                                g, hc, zt, CH)
                            func = AF.Tanh if g == 3 else AF.Sigmoid
                            nc.scalar.activation(
                                out=lgates[g][:, hc, :], in_=zt[:], func=func,
                                bias=bias_sb[:, g * 2 + hc:g * 2 + hc + 1])

                def s2():
                    nc.vector.tensor_mul(cA[:, :, nsl], i_t[:], u_t[:])
                    nc.scalar.activation(out=u_t[:], in_=cA[:, :, nsl],
                                         func=AF.Tanh)
                    nc.vector.tensor_mul(hA[:, :, nsl], o_t[:], u_t[:])
                    if ch == NCHUNK - 1:
                        for hc in range(2):
                            nc.sync.dma_start(out=out_d[hc, :, 0:LPC],
                                              in_=hA[:, hc, :])
                return s1, s2

            # ---- software-pipelined unit stream: leaves + internal levels ----
            # depth-3 pipeline; L2048 chunks issued (0,2,1,3) so each chunk's
            # child h (leaf s2 ch and ch+4) is already issued at depth 3
            units = [("leaf", ch, False, 3) for ch in range(NCHUNK)]
            cur = [hA, cA, hB, cB]
            lvl = 1
            n = LPC // 2
            while n >= CUT:
                h_src, c_src, h_dst, c_dst = cur
                # if the child level had <= 2 chunks, this level's first s1
                # reads h written by a pending s2 -> must flush the pipeline
                flush = (2 * n) // CH <= 2
                nch = max(1, n // CH)
                # depth 3 is safe only while each unit's inputs are >= 3
                # units back (leaves feeding L2048 in (0,2,1,3) order)
                order = (0, 2, 1, 3) if nch == 4 else range(nch)
                depth = 3 if nch == 4 else 2
                for k, ch in enumerate(order):
                    units.append(("int", (n, ch, h_src, c_src, h_dst, c_dst,
                                          lvl), flush and k == 0, depth))
                cur = [cur[2], cur[3], cur[0], cur[1]]
                lvl += 1
                n //= 2
            pending = []
            for u in units:
                if u[2]:
                    while pending:
                        pending.pop(0)()
                while len(pending) > u[3] - 1:
                    pending.pop(0)()
                s1, s2 = (unit_leaves(u[1]) if u[0] == "leaf"
                          else unit_internal(*u[1]))
                s1()
                pending.append(s2)
            for s2 in pending:
                s2()

    nc.compile()
    return nc, sizes, offs


class _ExecHandle:
    """Compiled SPMD executable with device-resident input support."""

    def __init__(self, nc):
        import jax
        from jax.sharding import Mesh, PartitionSpec
        try:
            from jax.experimental.shard_map import shard_map
        except ImportError:
            from jax.shard_map import shard_map
        from concourse import bass2jax, mybir

        bass2jax.install_neuronx_cc_hook()
        self.jax = jax
        partition_name = (nc.partition_id_tensor.name
                          if nc.partition_id_tensor else None)
        in_names, out_names, out_avals, zero_outs = [], [], [], []
        for alloc in nc.m.functions[0].allocations:
            if not isinstance(alloc, mybir.MemoryLocationSet):
                continue
            name = alloc.memorylocations[0].name
            if alloc.kind == "ExternalInput":
                if name != partition_name:
                    in_names.append(name)
            elif alloc.kind == "ExternalOutput":
                out_names.append(name)
                shape = tuple(alloc.tensor_shape)
                dtype = mybir.dt.np(alloc.dtype)
                out_avals.append(jax.core.ShapedArray(shape, dtype))
                zero_outs.append(np.zeros(shape, dtype))
        self.n_params = len(in_names)
        self.out_names = list(out_names)
        self.param_names = list(in_names)
        all_in_names = in_names + out_names
        if partition_name is not None:
            all_in_names.append(partition_name)
        self.out_avals = out_avals
        self.zero_outs = zero_outs

        def _body(*args):
            operands = list(args)
            if partition_name is not None:
                operands.append(bass2jax.partition_id_tensor())
            outs = bass2jax._bass_exec_p.bind(
                *operands,
                out_avals=tuple(out_avals),
                in_names=tuple(all_in_names),
                out_names=tuple(out_names),
                lowering_input_output_aliases=(),
                sim_require_finite=True,
                sim_require_nnan=True,
                nc=nc,
            )
            return tuple(outs)

        self._body = _body

        devices = jax.devices()[:NCORES]
        self.mesh = Mesh(np.asarray(devices), ("core",))
        n_ops = self.n_params + len(out_names)
        self.fn = jax.jit(shard_map(
            _body, mesh=self.mesh,
            in_specs=(PartitionSpec("core"),) * n_ops,
            out_specs=(PartitionSpec("core"),) * len(out_names),
            check_rep=False))

    def put_inputs(self, in_maps):
        import jax
        from jax.sharding import NamedSharding, PartitionSpec
        sh = NamedSharding(self.mesh, PartitionSpec("core"))
        ops = []
        for i, name in enumerate(self.param_names):
            arr = np.concatenate([np.asarray(m[name]) for m in in_maps], axis=0)
            ops.append(jax.device_put(arr, sh))
        for z in self.zero_outs:
            zz = np.zeros((NCORES * z.shape[0], *z.shape[1:]), z.dtype)
            ops.append(jax.device_put(zz, sh))
        return ops

    def run(self, ops):
        outs = self.fn(*ops)
        self.jax.block_until_ready(outs)
        return outs

    def results(self, outs):
        res = []
        for c in range(NCORES):
            d = {}
            for i, name in enumerate(self.out_names):
                a = np.asarray(outs[i])
                d[name] = a.reshape(NCORES, *self.out_avals[i].shape)[c]
            res.append(d)
        return res


def _sigmoid(z):
    with np.errstate(over="ignore"):
        return 1.0 / (1.0 + np.exp(-z))


_PROGRAM_CACHE = {}
_EXEC_CACHE = {}


def kernel(tokens, emb, Wx, Wl, Wr, b):
    global LAST_RESULTS, LAST_OPS
    tokens = np.asarray(tokens)
    emb = np.asarray(emb, dtype=np.float32)
    Wx = np.asarray(Wx, dtype=np.float32)
    Wl = np.asarray(Wl, dtype=np.float32)
    Wr = np.asarray(Wr, dtype=np.float32)
    b = np.asarray(b, dtype=np.float32)

    L = int(tokens.shape[0])
    LPC = L // NCORES
    mmdt = os.environ.get("TRNK_MM_DTYPE", "bfloat16")
    key = (LPC, mmdt)
    if key not in _PROGRAM_CACHE:
        _PROGRAM_CACHE[key] = _build_program(LPC, mmdt)
    nc, sizes, offs = _PROGRAM_CACHE[key]
    CH = 512

    wt_blob = _pack_weights(Wx, Wl, Wr)
    bias_blob = np.ascontiguousarray(
        b.reshape(4, 2, 128).transpose(2, 0, 1).reshape(128, 8)).astype(np.float32)

    x = emb[tokens]  # [L, 256] host gather (input sharding/staging)
    rp = _revperm(LPC)
    if mmdt == "bfloat16":
        import ml_dtypes
        wt_blob = wt_blob.astype(ml_dtypes.bfloat16)
        cast = lambda a: a.astype(ml_dtypes.bfloat16)  # noqa: E731
    else:
        wt_blob = _round_fp32r(wt_blob)
        cast = _round_fp32r
    in_maps = []
    for ci in range(NCORES):
        xc = x[ci * LPC:(ci + 1) * LPC][rp]                   # stored order
        # [128, NCHUNK, 2, CH]: chunk-major so each chunk DMA has big lines
        xblob = np.ascontiguousarray(
            xc.reshape(LPC // CH, CH, 2, 128).transpose(3, 0, 2, 1))
        in_maps.append({"x": cast(xblob), "wt": wt_blob, "bias": bias_blob})

    if key not in _EXEC_CACHE:
        _EXEC_CACHE[key] = _ExecHandle(nc)
    eh = _EXEC_CACHE[key]
    ops = eh.put_inputs(in_maps)
    outs = eh.run(ops)
    results = eh.results(outs)
    LAST_RESULTS = results
    LAST_OPS = ops

    # ---- host reassembly of device levels (global 32768 .. 2048) ----
    pieces = []
    for lvl, npc in enumerate(sizes):
        nglob = npc * NCORES
        rpl = _revperm(npc)
        lvlarr = np.empty((nglob, HIDDEN), np.float32)
        for ci in range(NCORES):
            if lvl < len(sizes) - 2:
                st = results[ci]["out"][:, :, offs[lvl]:offs[lvl] + npc]
                st = st.reshape(HIDDEN, npc)
            else:
                toff = 0 if npc == 512 else 512
                st = results[ci]["tail"][:, :, toff:toff + npc]
                st = st.transpose(1, 0, 2).reshape(HIDDEN, npc)
            lvlarr[ci * npc:(ci + 1) * npc] = st.T[rpl].astype(np.float32)
        pieces.append(lvlarr)

    # ---- host tail: global levels 1024 .. 1 from per-core (h,c) at CUT ----
    rpc = _revperm(CUT)
    nglob = CUT * NCORES
    h = np.empty((nglob, HIDDEN), np.float32)
    c = np.empty((nglob, HIDDEN), np.float32)
    for ci in range(NCORES):
        st = results[ci]["tail"][:, :, 512:512 + CUT]
        h[ci * CUT:(ci + 1) * CUT] = (
            st.transpose(1, 0, 2).reshape(HIDDEN, CUT).T[rpc].astype(np.float32))
        stc = results[ci]["cend"]                       # [128, 2, CUT] f32
        c[ci * CUT:(ci + 1) * CUT] = (
            stc.transpose(1, 0, 2).reshape(HIDDEN, CUT).T[rpc])

    # y = x @ W.T per gate; stack gates on columns: [in, 4*out]
    WlT = np.ascontiguousarray(Wl.transpose(2, 0, 1).reshape(HIDDEN, 4 * HIDDEN))
    WrT = np.ascontiguousarray(Wr.transpose(2, 0, 1).reshape(HIDDEN, 4 * HIDDEN))
    bfl = b.reshape(4 * HIDDEN)
    while h.shape[0] > 1:
        lh, rh = h[0::2], h[1::2]
        lc, rc = c[0::2], c[1::2]
        z = lh @ WlT + rh @ WrT + bfl                   # [n, 4H]
        i = _sigmoid(z[:, 0 * HIDDEN:1 * HIDDEN])
        f = _sigmoid(z[:, 1 * HIDDEN:2 * HIDDEN])
        o = _sigmoid(z[:, 2 * HIDDEN:3 * HIDDEN])
        u = np.tanh(z[:, 3 * HIDDEN:4 * HIDDEN])
        c = i * u + f * (lc + rc)
        h = o * np.tanh(c)
        pieces.append(h)
    return np.concatenate(pieces, axis=0)


# revision 24
# speedup vs baseline: 1.0626x; 1.0260x over previous
"""BinaryTreeLSTM on 8 Trainium2 NeuronCores (Bass/Tile).

Sharding: each core owns a contiguous subtree of 4096 leaves and reduces
it through the 4 largest internal levels (2048/1024/512/256 nodes per
core). The remaining global levels (1024 nodes down to the root, 2047
nodes = 0.25% of FLOPs) are latency-bound on device, so they are
finished on the host with BLAS from the per-core level-256 h/c — this
removes the AllGather collective and the serial small-level tail
entirely. Gate weights are replicated across cores.

Per-level node arrays are stored in bit-reversed node order on device so
each level's left/right children are the contiguous halves of the child
level; the host un-permutes when reassembling the natural level-order
output.

Device layout: hidden dim (256 = 2 chunks of 128) on the SBUF partition
axis, nodes on the free axis. Matmuls default to bf16 operands (fp32
PSUM accumulation; ~1e-3 output error) which halves input DMA and
enables fast weight loads; set TRNK_MM_DTYPE=float32r for the fp32r
path. Gate pre-activations use 2-bank PSUM tiles so one 1024-column
Scalar-engine activation (with the per-gate bias folded in) covers a
whole node chunk, amortizing the ~352-cycle per-instruction overhead.
"""

import os
import sys

import numpy as np

sys.path.insert(0, "/opt/trn_rl_repo")

HIDDEN = 256
NCORES = 8
CUT = 256       # smallest per-core level computed on device

# exposed for test harnesses
LAST_RESULTS = None
LAST_EXEC_NS = None
LAST_OPS = None


def _revperm(n):
    bits = n.bit_length() - 1
    r = np.arange(n)
    out = np.zeros(n, np.int64)
    for b in range(bits):
        out |= ((r >> b) & 1) << (bits - 1 - b)
    return out


def _w_tile_index(src, g, kc, hc):
    return ((src * 4 + g) * 2 + kc) * 2 + hc


def _round_fp32r(a):
    """Round fp32 values to the PE's fp32r format (1+8+11 bits, RNE)."""
    bits = np.ascontiguousarray(a, np.float32).view(np.uint32)
    odd = (bits >> np.uint32(12)) & np.uint32(1)
    bits = bits + np.uint32(0x7FF) + odd
    bits &= np.uint32(0xFFFFF000)
    return bits.view(np.float32)


def _pack_weights(Wx, Wl, Wr):
    # lhsT tile for (src, g, kc, hc): [p(contraction), m(out)] = W[g, hc*128+m, kc*128+p]
    tiles = []
    for W in (Wx, Wl, Wr):
        W4 = W.reshape(4, 2, 128, 2, 128)           # [g, hc, m, kc, p]
        tiles.append(W4.transpose(0, 3, 1, 4, 2))    # [g, kc, hc, p, m]
    allw = np.stack(tiles)                            # [3, 4, 2, 2, 128, 128]
    # -> [p, (s,g,kc,hc), m]
    blob = np.ascontiguousarray(allw.transpose(4, 0, 1, 2, 3, 5).reshape(128, 48, 128))
    return blob.astype(np.float32)


def _build_program(LPC, matmul_dtype_name="bfloat16"):
    from concourse import bacc, mybir, tile

    f32 = mybir.dt.float32
    mmdt = getattr(mybir.dt, matmul_dtype_name)
    bf16 = matmul_dtype_name == "bfloat16"
    sdt = mmdt if bf16 else f32
    CH = 512  # node-chunk (one z = CH fp32 = one PSUM bank)
    AF = mybir.ActivationFunctionType

    sizes = []
    n = LPC
    while n >= CUT:
        sizes.append(n)
        n //= 2
    offs = np.concatenate([[0], np.cumsum(sizes)]).astype(int)
    BIGTOT = int(offs[-3])          # leaves .. L1024 go to out_d
    NCHUNK = LPC // CH

    nc = bacc.Bacc("TRN2", target_bir_lowering=False, debug=False,
                   num_devices=NCORES)

    x_d = nc.dram_tensor("x", [128, NCHUNK, 2, CH], mmdt,
                         kind="ExternalInput").ap()
    wt_d = nc.dram_tensor("wt", [128, 48, 128], mmdt, kind="ExternalInput").ap()
    bias_d = nc.dram_tensor("bias", [128, 8], f32, kind="ExternalInput").ap()
    out_d = nc.dram_tensor("out", [2, 128, BIGTOT], sdt,
                           kind="ExternalOutput").ap()
    # h of the last two levels, flat [hc*768 + (L512 | L256)] per partition
    tail_d = nc.dram_tensor("tail", [128, 2, 768], sdt,
                            kind="ExternalOutput").ap()
    cend_d = nc.dram_tensor("cend", [128, 2, CUT], f32,
                            kind="ExternalOutput").ap()

    with tile.TileContext(nc) as tc:
        with tc.tile_pool(name="pp", bufs=1) as pp, \
             tc.tile_pool(name="zp", bufs=8, space="PSUM") as zp, \
             tc.tile_pool(name="gp", bufs=4) as gp:
            w_sb = pp.tile([128, 48, 128], mmdt, name="w_sb")
            bias_sb = pp.tile([128, 8], f32, name="bias_sb")
            hA = pp.tile([128, 2, LPC], sdt, name="hA")
            cA = pp.tile([128, 2, LPC], f32, name="cA")
            hB = pp.tile([128, 2, LPC // 2], sdt, name="hB")
            cB = pp.tile([128, 2, LPC // 2], f32, name="cB")
            x0_sb = pp.tile([128, 2, CH], mmdt, name="x0_sb")
            x1_sb = pp.tile([128, 2, CH], mmdt, name="x1_sb")
            x2_sb = pp.tile([128, 2, CH], mmdt, name="x2_sb")
            xr_sb = pp.tile([128, NCHUNK - 3, 2, CH], mmdt, name="xr_sb")
            tail_sb = pp.tile([128, 2, 768], sdt, name="tail_sb")
            cend_sb = pp.tile([128, 2, CUT], f32, name="cend_sb")

            # Inputs split across the 3 DMA-capable engine queues (one DGE
            # queue per engine — same-engine DMAs serialize). First-needed
            # pieces lead their queues: bias unblocks the first activation,
            # w16+x0 unblock the first matmul; x chunks are striped so each
            # arrives just before the leaf pipeline consumes it.
            nc.scalar.dma_start(out=w_sb[:, 0:16, :], in_=wt_d[:, 0:16, :])
            nc.scalar.dma_start(out=x1_sb[:], in_=x_d[:, 1])
            nc.scalar.dma_start(out=x2_sb[:], in_=x_d[:, 2])
            nc.sync.dma_start(out=x0_sb[:], in_=x_d[:, 0])
            nc.sync.dma_start(out=xr_sb[:], in_=x_d[:, 3:NCHUNK])
            nc.gpsimd.dma_start(out=bias_sb[:], in_=bias_d[:])
            nc.gpsimd.dma_start(out=w_sb[:, 16:48, :], in_=wt_d[:, 16:48, :])

            if bf16:
                cast_rhs = lambda ap: ap  # noqa: E731
            else:
                cast_rhs = lambda ap: ap.bitcast(mmdt)  # noqa: E731

            def mm(w_idx, rhs_ap, zt, start, stop):
                nc.tensor.matmul(zt, w_sb[:, w_idx, :], cast_rhs(rhs_ap),
                                 start=start, stop=stop)

            def zmm(srcs, g, hc, zt, m):
                """z[:, :m] (+)= sum over (src, kc, h-ap); <=512 cols per mm."""
                for half in range(0, m, 512):
                    hw = min(512, m - half)
                    first = True
                    for src, kc, ap in srcs:
                        mm(_w_tile_index(src, g, kc, hc),
                           ap[:, half:half + hw],
                           zt[:, half:half + hw], first,
                           (src, kc) == srcs[-1][:2])
                        first = False

            def unit_internal(n, ch, h_src, c_src, h_dst, c_dst, lvl):
                """One chunk of an internal level -> (stage1, stage2)."""
                nchunks = max(1, n // CH)
                m = min(n, CH)
                lsl = slice(ch * m, (ch + 1) * m)
                rsl = slice(n + ch * m, n + (ch + 1) * m)
                dsl = slice(ch * m, (ch + 1) * m)
                merge = n >= 2048  # hc-merged ops off the flush critical path
                i_t = gp.tile([128, 2, CH], f32, name="i_t")
                f_t = gp.tile([128, 2, CH], f32, name="f_t")
                o_t = gp.tile([128, 2, CH], f32, name="o_t")
                u_t = gp.tile([128, 2, CH], f32, name="u_t")
                s_t = gp.tile([128, 2, CH], f32, name="s_t")
                gates = {0: i_t, 1: f_t, 2: o_t, 3: u_t}

                def s1():
                    nc.gpsimd.tensor_add(s_t[:, :, :m],
                                         c_src[:, :, lsl],
                                         c_src[:, :, rsl])
                    for hc in range(2):
                        for g in (0, 3, 1, 2):
                            zt = zp.tile([128, CH], f32, name="zt")
                            zmm([(1, 0, h_src[:, 0, lsl]),
                                 (1, 1, h_src[:, 1, lsl]),
                                 (2, 0, h_src[:, 0, rsl]),
                                 (2, 1, h_src[:, 1, rsl])], g, hc, zt, m)
                            func = AF.Tanh if g == 3 else AF.Sigmoid
                            nc.scalar.activation(
                                out=gates[g][:, hc, :m], in_=zt[:, :m],
                                func=func,
                                bias=bias_sb[:, g * 2 + hc:g * 2 + hc + 1])

                def s2():
                    if merge:
                        # hc-merged [128, 2, m] APs halve the instruction count
                        nc.vector.tensor_mul(u_t[:, :, :m], i_t[:, :, :m],
                                             u_t[:, :, :m])
                        nc.vector.tensor_mul(s_t[:, :, :m], f_t[:, :, :m],
                                             s_t[:, :, :m])
                        nc.vector.tensor_add(c_dst[:, :, dsl], u_t[:, :, :m],
                                             s_t[:, :, :m])
                        nc.scalar.activation(out=i_t[:, :, :m],
                                             in_=c_dst[:, :, dsl],
                                             func=AF.Tanh)
                        nc.vector.tensor_mul(h_dst[:, :, dsl],
                                             o_t[:, :, :m], i_t[:, :, :m])
                    else:
                        # per-hc keeps two short parallel chains for the
                        # latency-critical flushed levels
                        for hc in range(2):
                            nc.vector.tensor_mul(u_t[:, hc, :m],
                                                 i_t[:, hc, :m],
                                                 u_t[:, hc, :m])
                            nc.vector.tensor_mul(s_t[:, hc, :m],
                                                 f_t[:, hc, :m],
                                                 s_t[:, hc, :m])
                            nc.vector.tensor_add(c_dst[:, hc, dsl],
                                                 u_t[:, hc, :m],
                                                 s_t[:, hc, :m])
                            nc.scalar.activation(out=i_t[:, hc, :m],
                                                 in_=c_dst[:, hc, dsl],
                                                 func=AF.Tanh)
                            nc.vector.tensor_mul(h_dst[:, hc, dsl],
                                                 o_t[:, hc, :m],
                                                 i_t[:, hc, :m])
                    if ch == nchunks - 1:
                        if n >= 1024:
                            for hc in range(2):
                                nc.sync.dma_start(
                                    out=out_d[hc, :, offs[lvl]:offs[lvl] + n],
                                    in_=h_dst[:, hc, :n])
                        else:
                            toff = 0 if n == 512 else 512
                            for hc in range(2):
                                nc.vector.tensor_copy(
                                    tail_sb[:, hc, toff:toff + n],
                                    h_dst[:, hc, :n])
                            if n == CUT:
                                for hc in range(2):
                                    nc.vector.tensor_copy(cend_sb[:, hc, :],
                                                          c_dst[:, hc, :n])
                                nc.sync.dma_start(out=tail_d[:], in_=tail_sb[:])
                                nc.sync.dma_start(out=cend_d[:], in_=cend_sb[:])
                return s1, s2

            def unit_leaves(ch):
                nsl = slice(ch * CH, (ch + 1) * CH)
                xc_t = ([x0_sb, x1_sb, x2_sb][ch] if ch < 3
                        else xr_sb[:, ch - 3])
                i_t = gp.tile([128, 2, CH], f32, name="i_t")
                o_t = gp.tile([128, 2, CH], f32, name="o_t")
                u_t = gp.tile([128, 2, CH], f32, name="u_t")
                lgates = {0: i_t, 2: o_t, 3: u_t}

                def s1():
                    for hc in range(2):
                        for g in (0, 3, 2):
                            zt = zp.tile([128, CH], f32, name="zt")
                            zmm([(0, 0, xc_t[:, 0]), (0, 1, xc_t[:, 1])],
                                g, hc, zt, CH)
                            func = AF.Tanh if g == 3 else AF.Sigmoid
                            nc.scalar.activation(
                                out=lgates[g][:, hc, :], in_=zt[:], func=func,
                                bias=bias_sb[:, g * 2 + hc:g * 2 + hc + 1])

                def s2():
                    nc.vector.tensor_mul(cA[:, :, nsl], i_t[:], u_t[:])
                    nc.scalar.activation(out=u_t[:], in_=cA[:, :, nsl],
                                         func=AF.Tanh)
                    nc.vector.tensor_mul(hA[:, :, nsl], o_t[:], u_t[:])
                    if ch == NCHUNK - 1:
                        for hc in range(2):
                            nc.sync.dma_start(out=out_d[hc, :, 0:LPC],
                                              in_=hA[:, hc, :])
                return s1, s2

            # ---- software-pipelined unit stream: leaves + internal levels ----
            # depth-3 pipeline; L2048 chunks issued (0,2,1,3) so each chunk's
            # child h (leaf s2 ch and ch+4) is already issued at depth 3
            units = [("leaf", ch, False, 3) for ch in range(NCHUNK)]
            cur = [hA, cA, hB, cB]
            lvl = 1
            n = LPC // 2
            while n >= CUT:
                h_src, c_src, h_dst, c_dst = cur
                # if the child level had <= 2 chunks, this level's first s1
                # reads h written by a pending s2 -> must flush the pipeline
                flush = (2 * n) // CH <= 2
                nch = max(1, n // CH)
                # depth 3 is safe only while each unit's inputs are >= 3
                # units back (leaves feeding L2048 in (0,2,1,3) order)
                order = (0, 2, 1, 3) if nch == 4 else range(nch)
                depth = 3 if nch == 4 else 2
                for k, ch in enumerate(order):
                    units.append(("int", (n, ch, h_src, c_src, h_dst, c_dst,
                                          lvl), flush and k == 0, depth))
                cur = [cur[2], cur[3], cur[0], cur[1]]
                lvl += 1
                n //= 2
            pending = []
            for u in units:
                if u[2]:
                    while pending:
                        pending.pop(0)()
                while len(pending) > u[3] - 1:
                    pending.pop(0)()
                s1, s2 = (unit_leaves(u[1]) if u[0] == "leaf"
                          else unit_internal(*u[1]))
                s1()
                pending.append(s2)
            for s2 in pending:
                s2()

    nc.compile()
    return nc, sizes, offs


class _ExecHandle:
    """Compiled SPMD executable with device-resident input support."""

    def __init__(self, nc):
        import jax
        from jax.sharding import Mesh, PartitionSpec
        try:
            from jax.experimental.shard_map import shard_map
        except ImportError:
            from jax.shard_map import shard_map
        from concourse import bass2jax, mybir

        bass2jax.install_neuronx_cc_hook()
        self.jax = jax
        partition_name = (nc.partition_id_tensor.name
                          if nc.partition_id_tensor else None)
        in_names, out_names, out_avals, zero_outs = [], [], [], []
        for alloc in nc.m.functions[0].allocations:
            if not isinstance(alloc, mybir.MemoryLocationSet):
                continue
            name = alloc.memorylocations[0].name
            if alloc.kind == "ExternalInput":
                if name != partition_name:
                    in_names.append(name)
            elif alloc.kind == "ExternalOutput":
                out_names.append(name)
                shape = tuple(alloc.tensor_shape)
                dtype = mybir.dt.np(alloc.dtype)
                out_avals.append(jax.core.ShapedArray(shape, dtype))
                zero_outs.append(np.zeros(shape, dtype))
        self.n_params = len(in_names)
        self.out_names = list(out_names)
        self.param_names = list(in_names)
        all_in_names = in_names + out_names
        if partition_name is not None:
            all_in_names.append(partition_name)
        self.out_avals = out_avals
        self.zero_outs = zero_outs

        def _body(*args):
            operands = list(args)
            if partition_name is not None:
                operands.append(bass2jax.partition_id_tensor())
            outs = bass2jax._bass_exec_p.bind(
                *operands,
                out_avals=tuple(out_avals),
                in_names=tuple(all_in_names),
                out_names=tuple(out_names),
                lowering_input_output_aliases=(),
                sim_require_finite=True,
                sim_require_nnan=True,
                nc=nc,
            )
            return tuple(outs)

        self._body = _body

        devices = jax.devices()[:NCORES]
        self.mesh = Mesh(np.asarray(devices), ("core",))
        n_ops = self.n_params + len(out_names)
        self.fn = jax.jit(shard_map(
            _body, mesh=self.mesh,
            in_specs=(PartitionSpec("core"),) * n_ops,
            out_specs=(PartitionSpec("core"),) * len(out_names),
            check_rep=False))

    def put_inputs(self, in_maps):
        import jax
        from jax.sharding import NamedSharding, PartitionSpec
        sh = NamedSharding(self.mesh, PartitionSpec("core"))
        ops = []
        for i, name in enumerate(self.param_names):
            arr = np.concatenate([np.asarray(m[name]) for m in in_maps], axis=0)
            ops.append(jax.device_put(arr, sh))
        for z in self.zero_outs:
            zz = np.zeros((NCORES * z.shape[0], *z.shape[1:]), z.dtype)
            ops.append(jax.device_put(zz, sh))
        return ops

    def run(self, ops):
        outs = self.fn(*ops)
        self.jax.block_until_ready(outs)
        return outs

    def results(self, outs):
        res = []
        for c in range(NCORES):
            d = {}
            for i, name in enumerate(self.out_names):
                a = np.asarray(outs[i])
                d[name] = a.reshape(NCORES, *self.out_avals[i].shape)[c]
            res.append(d)
        return res


def _sigmoid(z):
    with np.errstate(over="ignore"):
        return 1.0 / (1.0 + np.exp(-z))


_PROGRAM_CACHE = {}
_EXEC_CACHE = {}


def kernel(tokens, emb, Wx, Wl, Wr, b):
    global LAST_RESULTS, LAST_OPS
    tokens = np.asarray(tokens)
    emb = np.asarray(emb, dtype=np.float32)
    Wx = np.asarray(Wx, dtype=np.float32)
    Wl = np.asarray(Wl, dtype=np.float32)
    Wr = np.asarray(Wr, dtype=np.float32)
    b = np.asarray(b, dtype=np.float32)

    L = int(tokens.shape[0])
    LPC = L // NCORES
    mmdt = os.environ.get("TRNK_MM_DTYPE", "bfloat16")
    key = (LPC, mmdt)
    if key not in _PROGRAM_CACHE:
        _PROGRAM_CACHE[key] = _build_program(LPC, mmdt)
    nc, sizes, offs = _PROGRAM_CACHE[key]
    CH = 512

    wt_blob = _pack_weights(Wx, Wl, Wr)
    bias_blob = np.ascontiguousarray(
        b.reshape(4, 2, 128).transpose(2, 0, 1).reshape(128, 8)).astype(np.float32)

    x = emb[tokens]  # [L, 256] host gather (input sharding/staging)
    rp = _revperm(LPC)
    if mmdt == "bfloat16":
        import ml_dtypes
        wt_blob = wt_blob.astype(ml_dtypes.bfloat16)
        cast = lambda a: a.astype(ml_dtypes.bfloat16)  # noqa: E731
    else:
        wt_blob = _round_fp32r(wt_blob)
        cast = _round_fp32r
    in_maps = []
    for ci in range(NCORES):
        xc = x[ci * LPC:(ci + 1) * LPC][rp]                   # stored order
        # [128, NCHUNK, 2, CH]: chunk-major so each chunk DMA has big lines
        xblob = np.ascontiguousarray(
            xc.reshape(LPC // CH, CH, 2, 128).transpose(3, 0, 2, 1))
        in_maps.append({"x": cast(xblob), "wt": wt_blob, "bias": bias_blob})

    if key not in _EXEC_CACHE:
        _EXEC_CACHE[key] = _ExecHandle(nc)
    eh = _EXEC_CACHE[key]
    ops = eh.put_inputs(in_maps)
    outs = eh.run(ops)
    results = eh.results(outs)
    LAST_RESULTS = results
    LAST_OPS = ops

    # ---- host reassembly of device levels (global 32768 .. 2048) ----
    pieces = []
    for lvl, npc in enumerate(sizes):
        nglob = npc * NCORES
        rpl = _revperm(npc)
        lvlarr = np.empty((nglob, HIDDEN), np.float32)
        for ci in range(NCORES):
            if lvl < len(sizes) - 2:
                st = results[ci]["out"][:, :, offs[lvl]:offs[lvl] + npc]
                st = st.reshape(HIDDEN, npc)
            else:
                toff = 0 if npc == 512 else 512
                st = results[ci]["tail"][:, :, toff:toff + npc]
                st = st.transpose(1, 0, 2).reshape(HIDDEN, npc)
            lvlarr[ci * npc:(ci + 1) * npc] = st.T[rpl].astype(np.float32)
        pieces.append(lvlarr)

    # ---- host tail: global levels 1024 .. 1 from per-core (h,c) at CUT ----
    rpc = _revperm(CUT)
    nglob = CUT * NCORES
    h = np.empty((nglob, HIDDEN), np.float32)
    c = np.empty((nglob, HIDDEN), np.float32)
    for ci in range(NCORES):
        st = results[ci]["tail"][:, :, 512:512 + CUT]
        h[ci * CUT:(ci + 1) * CUT] = (
            st.transpose(1, 0, 2).reshape(HIDDEN, CUT).T[rpc].astype(np.float32))
        stc = results[ci]["cend"]                       # [128, 2, CUT] f32
        c[ci * CUT:(ci + 1) * CUT] = (
            stc.transpose(1, 0, 2).reshape(HIDDEN, CUT).T[rpc])

    # y = x @ W.T per gate; stack gates on columns: [in, 4*out]
    WlT = np.ascontiguousarray(Wl.transpose(2, 0, 1).reshape(HIDDEN, 4 * HIDDEN))
    WrT = np.ascontiguousarray(Wr.transpose(2, 0, 1).reshape(HIDDEN, 4 * HIDDEN))
    bfl = b.reshape(4 * HIDDEN)
    while h.shape[0] > 1:
        lh, rh = h[0::2], h[1::2]
        lc, rc = c[0::2], c[1::2]
        z = lh @ WlT + rh @ WrT + bfl                   # [n, 4H]
        i = _sigmoid(z[:, 0 * HIDDEN:1 * HIDDEN])
        f = _sigmoid(z[:, 1 * HIDDEN:2 * HIDDEN])
        o = _sigmoid(z[:, 2 * HIDDEN:3 * HIDDEN])
        u = np.tanh(z[:, 3 * HIDDEN:4 * HIDDEN])
        c = i * u + f * (lc + rc)
        h = o * np.tanh(c)
        pieces.append(h)
    return np.concatenate(pieces, axis=0)
